# revision 54
# baseline (speedup 1.0000x reference)
"""Trainium2 Bass kernel for nn_RandomDelayGwAC (gnn_message_passing).

Design (v2 — batched SWDGE gathers):
  - 256 independent replays; the heap schedule (proc/parent/valid) is a
    kernel input, so the host retimes all ~325k valid steps into ~220
    dependency rounds (every dep >= 2 rounds back) and assigns 32 runs to
    each of 8 cores (snake by DAG depth).
  - One fp16 HBM table row per step: [ns(256) | nm(128) | junk(128)] =
    1KB = two 512B "slots". Round inputs are fetched by a SINGLE
    dma_gather (InstDMAGatherAnt, SWDGE) per round with transpose=True:
    state reads slot 2r (ns), message reads slot 2r+1 (nm|junk); the
    transposed layout lands feature-major, so the 3+3 fp16 matmuls
    consume it directly with no PE input transposes.
  - int16 gather indices only span +-32767 slots, so each round's
    in-AP is a sliding window [lo, vis) over the table; the scheduler
    force-schedules items before their dependency leaves the ~16k-row
    window, and the [pred0|first_message] init block is re-copied into
    the row stream every ~8k rows so chain-start reads stay in-window.
  - Window slicing doubles as precise RAW/WAR ranges for the Tile
    scheduler: round l gathers only rows written <= l-2, so gathers
    prefetch a full round ahead of compute.
  - fp16 matmuls run at 1 row/cycle with fp32 PSUM accumulation
    (fp16's 10-bit mantissa matches f32r); end-to-end error ~2e-4.
"""

import numpy as np

N = 256
T = 1280
H = 256
MSG = 128
NCORES = 8
P = 128
ROWW = 512          # table row width in f16 elems (1KB): [ns 256|nm 128|junk]
SLOT = 256          # slot = 256 f16 = 512B; row = 2 slots
WIN_ROWS = 16350    # max rows per gather window (2*16350+1 < 32767)
URG_ROWS = 15200    # force-schedule when dep row older than this
RF_TRIG = 14200     # refresh a row once this old if it has pending readers
INIT_GAP = 8000     # re-copy init block after this many rows
CAP = 256           # soft width cap per round
HARDCAP = 384       # hard cap (3 blocks)
SPACING = 2


# ----------------------------------------------------------------------------
# host-side scheduling
# ----------------------------------------------------------------------------

def _build_schedule(proc, parent, valid):
    import heapq

    n, t_max = proc.shape
    ar = np.arange(n)

    # state-chain predecessor per item; final item per run
    prev_item = np.full((n, t_max), -1, np.int64)
    last_item = np.full((n, N), -1, np.int64)
    for t in range(t_max):
        v = valid[:, t]
        node = proc[:, t]
        prev_item[:, t] = np.where(v, last_item[ar, node], -1)
        last_item[ar, node] = np.where(v, ar * t_max + t, last_item[ar, node])
    fin_item = last_item[ar, ar]

    # downstream depth (longest chain below) per item, reverse pass
    dd = np.zeros((n, t_max), np.int32)
    for t in range(t_max - 1, -1, -1):
        v = valid[:, t]
        d1 = prev_item[:, t]
        d = dd[:, t] + 1
        has1 = v & (d1 >= 0)
        r1 = d1 // t_max
        t1 = d1 % t_max
        cur = dd[r1[has1], t1[has1]]
        dd[r1[has1], t1[has1]] = np.maximum(cur, d[has1])
        p1 = parent[:, t]
        has2 = v & (p1 >= 0)
        cur = dd[ar[has2], p1[has2]]
        dd[ar[has2], p1[has2]] = np.maximum(cur, d[has2])

    # forward depth for run balance
    lev = np.zeros((n, t_max), np.int32)
    last_lev = np.zeros((n, N), np.int32)
    for t in range(t_max):
        v = valid[:, t]
        node = proc[:, t]
        p = parent[:, t]
        lp = np.where(p >= 0, lev[ar, np.maximum(p, 0)], 0)
        ln = last_lev[ar, node]
        lev[:, t] = np.where(v, 1 + np.maximum(lp, ln), 0)
        last_lev[ar, node] = np.where(v, lev[:, t], last_lev[ar, node])
    depths = lev.max(axis=1)

    order = np.argsort(-depths, kind="stable")
    core_of = np.zeros(n, np.int32)
    for k, r in enumerate(order):
        c = k % (2 * NCORES)
        core_of[r] = c if c < NCORES else 2 * NCORES - 1 - c
    runs_of = [np.where(core_of == c)[0] for c in range(NCORES)]

    dd_f = dd.ravel()
    prev_f = prev_item.ravel()
    parent_f = parent.ravel()
    proc_f = proc.ravel()
    valid_f = valid.ravel()

    # per-core dependency graphs
    cores = []
    for c in range(NCORES):
        children = {}
        indeg = {}
        e_rnd = {}
        heap0 = []
        total = 0
        for ridx, r in enumerate(runs_of[c]):
            base = r * t_max
            tt = np.where(valid_f[base:base + t_max])[0]
            for t in tt:
                it = base + t
                total += 1
                deg = 0
                d1 = prev_f[it]
                if d1 >= 0:
                    children.setdefault(d1, []).append(it)
                    deg += 1
                pa = parent_f[it]
                if pa >= 0:
                    children.setdefault(base + pa, []).append(it)
                    deg += 1
                indeg[it] = deg
                e_rnd[it] = 1
                if deg == 0:
                    heapq.heappush(heap0, (1, -int(dd_f[it]), it))
        cores.append({
            "children": children, "indeg": indeg, "e_rnd": e_rnd,
            "H": heap0, "RH": [], "total": total, "done": 0,
            "row_of": {},        # item -> latest copy row
            "rowhist": {},       # item -> [(round, row), ...] copies
            "minrow": {},        # ready item -> min dep row (for urgency)
            "WH": [],            # watch heap: (dep_row, dep_item, reader)
        })

    # global round loop with shared row allocation
    rows = 0
    copies = []          # absolute row of each init-copy block
    rounds = []          # per round: dict(nb, row0, lo, vis, per-core items)
    row_end_hist = []    # rows after round p (1-based index p-1)

    def place_copy():
        nonlocal rows
        copies.append(rows)
        rows += N

    place_copy()  # initial init block at rows [0, 256)

    p = 0
    while any(co["done"] < co["total"] for co in cores):
        p += 1
        if rows - copies[-1] > INIT_GAP:
            place_copy()
        # visible rows: everything allocated up to end of round p-2
        if p - 2 >= 1:
            vis = row_end_hist[p - 3]
        else:
            vis = N  # only initial init block
        urg_cut = rows - URG_ROWS
        rf_cut = rows - RF_TRIG

        # refresh pass: re-copy aged rows that still have pending readers
        refresh_of = []     # per core: list of old absolute rows
        for co in cores:
            WH = co["WH"]
            ref = []
            seen = set()
            while WH and WH[0][0] < rf_cut and len(ref) < 2 * P:
                erow, dep, ch = heapq.heappop(WH)
                if ch in co["row_of"]:
                    continue                      # reader already scheduled
                if co["indeg"][ch] == 0:
                    continue  # ready: the urgency path forces it in-window
                cur = co["row_of"][dep]
                if cur != erow:
                    # stale: a fresher copy exists; re-arm on it
                    heapq.heappush(WH, (cur, dep, ch))
                    continue
                if dep in seen:
                    continue                      # refreshed this round below
                seen.add(dep)
                ref.append((dep, erow))
            refresh_of.append(ref)
        nref = max(len(r) for r in refresh_of)
        ref_row0 = rows
        ref_blk = 0
        if nref:
            assert nref <= 2 * P, (p, nref)
            ref_blk = -(-nref // P)
            for ci, co in enumerate(cores):
                for k, (dep, erow) in enumerate(refresh_of[ci]):
                    nr = ref_row0 + k
                    co["row_of"][dep] = nr
                    co["rowhist"][dep].append((p, nr))
                    # re-arm watches for any remaining pending readers
                    for ch in co["children"].get(dep, ()):
                        if ch not in co["row_of"]:
                            heapq.heappush(co["WH"], (nr, dep, ch))
            rows += ref_blk * P

        take_of = []
        for co in cores:
            Hh, RH = co["H"], co["RH"]
            while Hh and Hh[0][0] <= p:
                _, nd, it = heapq.heappop(Hh)
                # min dep row for urgency (deps already have rows)
                mr = rows + (1 << 30)
                d1 = prev_f[it]
                if d1 >= 0:
                    mr = min(mr, co["row_of"][d1])
                pa = parent_f[it]
                if pa >= 0:
                    base = (it // t_max) * t_max
                    mr = min(mr, co["row_of"][base + pa])
                heapq.heappush(RH, (nd, it))
                co["minrow"][it] = mr
            # urgent items first (stale heap entries skipped lazily)
            big = 1 << 62
            urgent = [it for (_, it) in RH
                      if it not in co["row_of"]
                      and co["minrow"].get(it, big) < urg_cut]
            take_of.append(urgent)
        # width = max over cores, quantized
        worst = max(len(co["RH"]) for co in cores)
        nurg = max(len(u) for u in take_of)
        want = min(worst, CAP)
        if want > P:
            want = (want // P) * P
        want = max(want, nurg)
        assert want <= HARDCAP + P, (p, want, nurg)
        nb = max(1, -(-want // P))
        W_pad = nb * P

        row0 = rows
        items_of = []
        for ci, co in enumerate(cores):
            RH = co["RH"]
            urgent = set(take_of[ci])
            take = list(take_of[ci])
            while RH and len(take) < want:
                nd, it = heapq.heappop(RH)
                if it in co["row_of"] or it in urgent:
                    continue  # stale entry or already taken as urgent
                take.append(it)
            for k, it in enumerate(take):
                co["row_of"][it] = row0 + k
                co["rowhist"][it] = [(p, row0 + k)]
                co["minrow"].pop(it, None)
                for ch in co["children"].get(it, ()):
                    if co["e_rnd"][ch] < p + SPACING:
                        co["e_rnd"][ch] = p + SPACING
                    co["indeg"][ch] -= 1
                    if co["indeg"][ch] == 0:
                        heapq.heappush(co["H"],
                                       (co["e_rnd"][ch], -int(dd_f[ch]), ch))
                    heapq.heappush(co["WH"], (row0 + k, it, ch))
            co["done"] += len(take)
            items_of.append(take)
        rows += W_pad
        row_end_hist.append(rows)
        rounds.append({"nb": nb, "row0": row0, "vis": vis,
                       "items_of": items_of,
                       "nref": nref, "ref_row0": ref_row0, "ref_blk": ref_blk,
                       "refresh_of": refresh_of})

    R = len(rounds)
    TOTROWS = rows

    # second pass: per-round lo (min read row) and idx arrays
    # init-copy visible to round p: largest copy fully below vis
    def copy_row_for(co, it, pi):
        # latest copy of item `it` visible to (1-based) round pi+1
        hist = co["rowhist"][it]
        best = hist[0][1]
        for rnd, row in hist[1:]:
            if rnd <= (pi + 1) - SPACING:
                best = row
        return best

    sidx = [[] for _ in range(NCORES)]   # per core: flat list of int16 vals
    col0 = []                            # per round: start column (in idx/16)
    lo_arr = np.zeros(R, np.int64)
    vis_arr = np.zeros(R, np.int64)
    nb_arr = np.zeros(R, np.int64)
    row0_arr = np.zeros(R, np.int64)
    refcol0 = np.full(R, -1, np.int64)
    refblk_arr = np.zeros(R, np.int64)
    refrow0_arr = np.zeros(R, np.int64)
    fr_cols = [[] for _ in range(NCORES)]  # per core: int32 col arrays [128]
    ccol = 0
    rcol = 0
    for pi, rd in enumerate(rounds):
        vis = rd["vis"]
        cvis = max(x for x in copies if x + N <= vis)
        nb = rd["nb"]
        W_pad = nb * P
        # refresh blocks for this round
        if rd["ref_blk"]:
            refcol0[pi] = rcol
            refblk_arr[pi] = rd["ref_blk"]
            refrow0_arr[pi] = rd["ref_row0"]
            for b in range(rd["ref_blk"]):
                for ci in range(NCORES):
                    colv = np.zeros(128, np.int32)
                    ref = rd["refresh_of"][ci]
                    for k in range(b * P, min((b + 1) * P, len(ref))):
                        erow = ref[k][1]
                        assert erow < vis, (pi, erow, vis)
                        colv[k - b * P] = erow
                    fr_cols[ci].append(colv)
                rcol += 1
        # compute lo over all cores' reads
        lo = cvis
        reads_of = []
        for ci, co in enumerate(cores):
            st = np.zeros(W_pad, np.int64)
            ms = np.zeros(W_pad, np.int64)
            st[:] = cvis          # padding -> init row (always in-window)
            ms[:] = cvis
            for k, it in enumerate(rd["items_of"][ci]):
                d1 = prev_f[it]
                if d1 >= 0:
                    srow = copy_row_for(co, d1, pi)
                else:
                    srow = cvis + proc_f[it]      # init: pred0[node]
                pa = parent_f[it]
                if pa >= 0:
                    base = (it // t_max) * t_max
                    mrow = copy_row_for(co, base + pa, pi)
                else:
                    mrow = cvis                    # init: first_message
                st[k] = srow
                ms[k] = mrow
                lo = min(lo, srow, mrow)
            reads_of.append((st, ms))
        assert vis - lo <= WIN_ROWS, (pi, lo, vis, vis - lo)
        lo_arr[pi] = lo
        vis_arr[pi] = vis
        nb_arr[pi] = nb
        row0_arr[pi] = rd["row0"]
        col0.append(ccol)
        ccol += (2 * W_pad) // 16
        for ci in range(NCORES):
            st, ms = reads_of[ci]
            rel = np.concatenate([2 * (st - lo), 2 * (ms - lo) + 1])
            assert rel.min() >= 0 and rel.max() <= 32767, (
                pi, rel.min(), rel.max())
            sidx[ci].append(rel.astype(np.int16))

    TOTC = ccol
    NREFC = max(1, rcol)
    fridx_dev = np.zeros((NCORES, 128, NREFC), np.int32)
    for ci in range(NCORES):
        for j, colv in enumerate(fr_cols[ci]):
            fridx_dev[ci, :, j] = colv
    # device idx tensor [128, TOTC]: idx k of round at partition k%16
    # (replicated over the 8 groups of 16), column col0 + k//16
    sidx_dev = np.zeros((NCORES, 128, TOTC), np.int16)
    for ci in range(NCORES):
        for pi in range(R):
            rel = sidx[ci][pi]
            nidx = len(rel)
            seg = rel.reshape(nidx // 16, 16).T   # [16, nidx/16]
            sidx_dev[ci, :, col0[pi]:col0[pi] + nidx // 16] = np.tile(
                seg, (8, 1))

    # final rows (absolute slot index = 2*row) per core
    fidx_dev = np.zeros((NCORES, 128, 1), np.int32)
    for ci, co in enumerate(cores):
        for ridx, r in enumerate(runs_of[ci]):
            fi = fin_item[r]
            if fi >= 0 and fi in co["rowhist"]:
                fidx_dev[ci, ridx, 0] = co["rowhist"][fi][0][1]
            else:
                fidx_dev[ci, ridx, 0] = r  # pred0 row in init0

    return {
        "R": R,
        "TOTROWS": TOTROWS,
        "TOTC": TOTC,
        "NREFC": NREFC,
        "copies": copies,
        "nb": nb_arr,
        "row0": row0_arr,
        "lo": lo_arr,
        "vis": vis_arr,
        "col0": np.asarray(col0, np.int64),
        "refcol0": refcol0,
        "refblk": refblk_arr,
        "refrow0": refrow0_arr,
        "sidx": sidx_dev,
        "fidx": fidx_dev,
        "fridx": fridx_dev,
        "runs_of": runs_of,
    }


# ----------------------------------------------------------------------------
# numpy simulator (mirrors the device program; for offline validation)
# ----------------------------------------------------------------------------

def _simulate(sched, core, pred0, fmsg, Ws, bs, Wm, bm):
    Ws16 = Ws.astype(np.float16).astype(np.float32)
    Wm16 = Wm.astype(np.float16).astype(np.float32)
    bs = bs.astype(np.float16).astype(np.float32)
    bm = bm.astype(np.float16).astype(np.float32)
    R = sched["R"]
    tabS = np.zeros((sched["TOTROWS"], SLOT), np.float16)
    tabM = np.zeros((sched["TOTROWS"], SLOT), np.float16)
    for cp in sched["copies"]:
        tabS[cp:cp + N] = pred0.astype(np.float16)
        tabM[cp:cp + N, :MSG] = fmsg.astype(np.float16)
    sidx = sched["sidx"][core]
    fridx = sched["fridx"][core]
    for pi in range(R):
        # refresh copies first
        for b in range(int(sched["refblk"][pi])):
            col = int(sched["refcol0"][pi]) + b
            old = fridx[:, col].astype(np.int64)
            dst = int(sched["refrow0"][pi]) + b * P
            tabS[dst:dst + P] = tabS[old]
            tabM[dst:dst + P] = tabM[old]
        nb = int(sched["nb"][pi])
        W_pad = nb * P
        nidx = 2 * W_pad
        c0 = int(sched["col0"][pi])
        rel = sidx[:16, c0:c0 + nidx // 16].T.reshape(-1).astype(np.int64)
        rws = sched["lo"][pi] + rel
        state = tabS[rws[:W_pad]].astype(np.float32)       # ns rows
        msg = tabM[rws[W_pad:], :MSG].astype(np.float32)   # nm part
        inp = np.concatenate([state, msg], axis=1)         # [W_pad, 384]
        ns = np.maximum(inp @ Ws16.T + bs, 0.0)
        ns16 = ns.astype(np.float16).astype(np.float32)
        nm = np.concatenate([ns16, msg], axis=1) @ Wm16.T + bm
        row0 = int(sched["row0"][pi])
        tabS[row0:row0 + W_pad] = ns16.astype(np.float16)
        tabM[row0:row0 + W_pad, :MSG] = nm.astype(np.float16)
    fidx = sched["fidx"][core]
    return tabS[fidx[:, 0]].astype(np.float32)             # [128, 256]


# ----------------------------------------------------------------------------
# device program
# ----------------------------------------------------------------------------

_PROGRAM_CACHE = {}


def _build_program(sched, prefetch=2):
    import concourse.bass as bass
    import concourse.mybir as mybir
    import concourse.tile as tile
    from concourse import bacc, library_config
    from concourse.masks import make_identity

    f32 = mybir.dt.float32
    f16 = mybir.dt.float16
    i16 = mybir.dt.int16
    i32 = mybir.dt.int32

    nc = bacc.Bacc("TRN2", target_bir_lowering=False, debug=False,
                   enable_asserts=False, num_swdge_queues=4,
                   dynamic_dma_scratch_size=65536)

    R = sched["R"]
    TOTROWS = sched["TOTROWS"]
    TOTC = sched["TOTC"]
    nb_arr = sched["nb"]
    MAXNB = int(nb_arr.max())

    init_d = nc.dram_tensor("initrows", (N, ROWW), f16, kind="ExternalInput")
    ws_d = nc.dram_tensor("wsT", (P, 3 * H), f16, kind="ExternalInput")
    wm_d = nc.dram_tensor("wmT", (P, 3 * MSG), f16, kind="ExternalInput")
    bs_d = nc.dram_tensor("bsrow", (1, H), f16, kind="ExternalInput")
    bm_d = nc.dram_tensor("bmrow", (1, MSG), f16, kind="ExternalInput")
    sidx_d = nc.dram_tensor("sidx", (P, TOTC), i16, kind="ExternalInput")
    fidx_d = nc.dram_tensor("fidx", (P, 1), i32, kind="ExternalInput")
    fridx_d = nc.dram_tensor("fridx", (P, sched["NREFC"]), i32,
                             kind="ExternalInput")
    diag_d = nc.dram_tensor("diag", (P, H), f16, kind="ExternalOutput")

    tab = nc.dram_tensor("tab", (TOTROWS, ROWW), f16, kind="Internal")

    relu = mybir.ActivationFunctionType.Relu
    ident_act = mybir.ActivationFunctionType.Identity

    with tile.TileContext(nc) as tc:
        with (
            tc.tile_pool(name="const", bufs=1) as cpool,
            tc.tile_pool(name="rio", bufs=8) as riopool,
            tc.tile_pool(name="work", bufs=8) as wpool,
            tc.tile_pool(name="psA", bufs=3, space="PSUM") as psA,
            tc.tile_pool(name="psB", bufs=2, space="PSUM") as psB,
            tc.tile_pool(name="psF", bufs=1, space="PSUM") as psF,
        ):
            nc.gpsimd.load_library(library_config.mlp)

            ident_f = cpool.tile([P, P], f32)
            make_identity(nc, ident_f[:])
            ident = cpool.tile([P, P], f16)
            nc.vector.tensor_copy(ident[:], ident_f[:])
            ones = cpool.tile([1, P], f16)
            nc.vector.memset(ones[:], 1.0)

            ws_sb = cpool.tile([P, 3 * H], f16)
            nc.sync.dma_start(ws_sb[:], ws_d.ap()[:])
            wm_sb = cpool.tile([P, 3 * MSG], f16)
            nc.sync.dma_start(wm_sb[:], wm_d.ap()[:])
            bs_sb = cpool.tile([1, H], f16)
            nc.sync.dma_start(bs_sb[:], bs_d.ap()[:])
            bm_sb = cpool.tile([1, MSG], f16)
            nc.sync.dma_start(bm_sb[:], bm_d.ap()[:])

            sidx_sb = cpool.tile([P, TOTC], i16)
            nc.sync.dma_start(sidx_sb[:], sidx_d.ap()[:])
            fidx_sb = cpool.tile([P, 1], i32)
            nc.sync.dma_start(fidx_sb[:], fidx_d.ap()[:])
            fridx_sb = cpool.tile([P, sched["NREFC"]], i32)
            nc.sync.dma_start(fridx_sb[:], fridx_d.ap()[:])

            # init block -> SBUF staging -> every copy position
            st0 = cpool.tile([P, ROWW], f16)
            st1 = cpool.tile([P, ROWW], f16)
            nc.sync.dma_start(st0[:], init_d.ap()[0:P, :])
            nc.sync.dma_start(st1[:], init_d.ap()[P:N, :])
            for cp in sched["copies"]:
                nc.sync.dma_start(tab.ap()[cp:cp + P, :], st0[:])
                nc.sync.dma_start(tab.ap()[cp + P:cp + N, :], st1[:])

            import os as _os
            use_trig = bool(int(_os.environ.get("KTRIG", "0")))
            trig_sems = [nc.alloc_semaphore(f"gsem{i}") for i in range(8)] \
                if use_trig else None

            def emit_gather(pi, prepare=False):
                nb = int(nb_arr[pi])
                W_pad = nb * P
                nidx = 2 * W_pad
                lo = int(sched["lo"][pi])
                vis = int(sched["vis"][pi])
                c0 = int(sched["col0"][pi])
                in_ap = tab.ap()[lo:vis, :].rearrange(
                    "r (a b) -> (r a) b", b=SLOT)
                in_t = riopool.tile([P, 2 * 2 * MAXNB * P], f16, tag="in_t")
                out_ap = in_t[:, 0:2 * nidx].rearrange(
                    "p (a b) -> p a b", a=2)
                if prepare:
                    # desc-gen now (unconstrained); data deps defer to the
                    # matching trigger_dma on the same queue
                    nc.gpsimd.dma_gather(
                        out_ap, in_ap, sidx_sb[:, c0:c0 + nidx // 16],
                        nidx, nidx, SLOT, transpose=True,
                        queue_num=1,
                        prepare_only=True, sem=trig_sems[1])
                else:
                    nc.gpsimd.dma_gather(
                        out_ap, in_ap, sidx_sb[:, c0:c0 + nidx // 16],
                        nidx, nidx, SLOT, transpose=True, queue_num=pi % 4)
                return in_t

            def emit_refresh(l):
                for b in range(int(sched["refblk"][l])):
                    col = int(sched["refcol0"][l]) + b
                    vis_l = int(sched["vis"][l])
                    rtile = wpool.tile([P, ROWW], f16, tag="rtile")
                    nc.gpsimd.indirect_dma_start(
                        out=rtile[:], out_offset=None,
                        in_=tab.ap()[0:vis_l, :],
                        in_offset=bass.IndirectOffsetOnAxis(
                            ap=fridx_sb[:, col:col + 1], axis=0),
                    )
                    dst = int(sched["refrow0"][l]) + b * P
                    nc.sync.dma_start(tab.ap()[dst:dst + P, :], rtile[:])

            # software-pipelined rounds: body l emits nm-phase(l-1) (incl the
            # table write), then refresh(l)+gather(l+1), then ns-phase(l) —
            # so each engine queue always has non-head-blocked work from two
            # adjacent (independent) rounds.
            def ns_phase(l, in_t):
                nb = int(nb_arr[l])
                W_pad = nb * P
                nidx = 2 * W_pad
                out_rt = riopool.tile([P, MAXNB * ROWW], f16, tag="out_rt")
                ns_list = []
                for j in range(nb):
                    # chunk cols inside in_t 2D layout: (a, k) at a*nidx + k
                    s0 = j * P              # state chunk a=0
                    s1 = nidx + j * P       # state chunk a=1
                    mg = W_pad + j * P      # msg chunk a=0
                    ns_ps = psA.tile([P, H], f32, tag="ns_ps", space="PSUM")
                    nc.tensor.matmul(ns_ps[:], lhsT=ones[:],
                                     rhs=bs_sb[:], start=True, stop=False)
                    for k, off in enumerate((s0, s1, mg)):
                        nc.tensor.matmul(
                            ns_ps[:],
                            lhsT=in_t[:, off:off + P],
                            rhs=ws_sb[:, k * H:(k + 1) * H],
                            start=False, stop=(k == 2))
                    oo = j * ROWW
                    nc.scalar.activation(out_rt[:, oo:oo + H], ns_ps[:], relu)
                    ns_list.append((j, mg))
                # ns halves are final: write them now, one round before the
                # nm halves — only the nm write remains on the critical
                # write->gather dependency chain
                row0 = int(sched["row0"][l])
                nc.sync.dma_start(
                    tab.ap()[row0:row0 + nb * P, 0:H].rearrange(
                        "(a p) w -> p a w", p=P),
                    out_rt[:, :nb * ROWW].rearrange(
                        "p (a w) -> p a w", w=ROWW)[:, :, 0:H])
                return out_rt, ns_list

            def nm_phase(l, in_t, out_rt, ns_list):
                nb = int(nb_arr[l])
                nsTs = []
                for j, mg in ns_list:
                    oo = j * ROWW
                    nsT_ps = psB.tile([P, H], f16, tag="nsT_ps", space="PSUM")
                    for k in range(2):
                        nc.tensor.transpose(
                            out=nsT_ps[:, k * P:(k + 1) * P],
                            in_=out_rt[:, oo + k * P:oo + (k + 1) * P],
                            identity=ident[:])
                    nsT = wpool.tile([P, H], f16, tag="nsT")
                    nc.vector.tensor_copy(nsT[:], nsT_ps[:])
                    nsTs.append(nsT)
                for (j, mg), nsT in zip(ns_list, nsTs):
                    oo = j * ROWW
                    nm_ps = psB.tile([P, MSG], f32, tag="nm_ps", space="PSUM")
                    nc.tensor.matmul(nm_ps[:], lhsT=ones[:],
                                     rhs=bm_sb[:], start=True, stop=False)
                    for k in range(3):
                        lhsT = (nsT[:, k * P:(k + 1) * P] if k < 2
                                else in_t[:, mg:mg + P])
                        nc.tensor.matmul(
                            nm_ps[:], lhsT=lhsT,
                            rhs=wm_sb[:, k * MSG:(k + 1) * MSG],
                            start=False, stop=(k == 2))
                    nc.scalar.activation(out_rt[:, oo + H:oo + H + MSG],
                                         nm_ps[:], ident_act)
                row0 = int(sched["row0"][l])
                nc.sync.dma_start(
                    tab.ap()[row0:row0 + nb * P, H:ROWW].rearrange(
                        "(a p) w -> p a w", p=P),
                    out_rt[:, :nb * ROWW].rearrange(
                        "p (a w) -> p a w", w=ROWW)[:, :, H:ROWW])

            if use_trig:
                # prepare descriptors PLEAD rounds early; trigger in-place.
                # Per SWDGE queue (4, round-robin) preps/triggers alternate.
                PLEAD = 1   # one outstanding prep on queue 1
                tiles = {}
                for q in range(min(PLEAD, R)):
                    tiles[q] = emit_gather(q, prepare=True)
                carry = None
                for l in range(R):
                    if carry is not None:
                        nm_phase(*carry)
                    nc.gpsimd.trigger_dma(count=None, queue_num=1)
                    if l + PLEAD < R:
                        tiles[l + PLEAD] = emit_gather(l + PLEAD,
                                                       prepare=True)
                    emit_refresh(l)
                    out_rt, ns_list = ns_phase(l, tiles[l])
                    carry = (l, tiles.pop(l), out_rt, ns_list)
                nm_phase(*carry)
            else:
                in_t0 = emit_gather(0)
                carry = None     # (l-1, in_t, out_rt, ns_list)
                cur_in = in_t0
                for l in range(R):
                    if carry is not None:
                        nm_phase(*carry)
                    nxt = emit_gather(l + 1) if l + 1 < R else None
                    emit_refresh(l)
                    out_rt, ns_list = ns_phase(l, cur_in)
                    carry = (l, cur_in, out_rt, ns_list)
                    cur_in = nxt
                nm_phase(*carry)

            dout = wpool.tile([P, H], f16, tag="dout")
            nc.gpsimd.indirect_dma_start(
                out=dout[:], out_offset=None,
                in_=tab.ap()[:, :].rearrange("r (a b) -> (r a) b", b=SLOT),
                in_offset=bass.IndirectOffsetOnAxis(ap=fidx_sb[:, :1], axis=0),
            )
            nc.sync.dma_start(diag_d.ap()[:], dout[:])

    nc.compile()
    return nc


# ----------------------------------------------------------------------------
# entry point
# ----------------------------------------------------------------------------

def kernel(x, first_message, We, be, Ws, bs, Wm, bm, Wd, bd, proc, parent, valid):
    from concourse import bass_utils

    x = np.asarray(x, np.float32)
    first_message = np.asarray(first_message, np.float32)
    We = np.asarray(We, np.float32)
    be = np.asarray(be, np.float32)
    Ws = np.asarray(Ws, np.float32)
    bs = np.asarray(bs, np.float32)
    Wm = np.asarray(Wm, np.float32)
    bm = np.asarray(bm, np.float32)
    Wd = np.asarray(Wd, np.float32)
    bd = np.asarray(bd, np.float32)
    proc = np.asarray(proc, np.int32)
    parent = np.asarray(parent, np.int32)
    valid = np.asarray(valid, bool)

    import hashlib
    h = hashlib.sha1()
    for k in (proc.tobytes(), parent.tobytes(), valid.tobytes()):
        h.update(k)
    skey = h.hexdigest()
    if skey in _PROGRAM_CACHE:
        sched, nc = _PROGRAM_CACHE[skey]
    else:
        sched = _build_schedule(proc, parent, valid)
        nc = _build_program(sched)
        _PROGRAM_CACHE[skey] = (sched, nc)

    pred0 = (x @ We.T + be).astype(np.float32)

    # weight tiles: contraction chunk k on partitions
    WsT = np.ascontiguousarray(
        Ws.T.reshape(3, P, H).transpose(1, 0, 2).reshape(P, 3 * H)
    ).astype(np.float16)
    WmT = np.ascontiguousarray(
        Wm.T.reshape(3, P, MSG).transpose(1, 0, 2).reshape(P, 3 * MSG)
    ).astype(np.float16)

    init_rows = np.zeros((N, ROWW), np.float16)
    init_rows[:, :H] = pred0.astype(np.float16)
    init_rows[:, H:H + MSG] = first_message.reshape(1, MSG).astype(np.float16)

    in_maps = []
    for c in range(NCORES):
        in_maps.append({
            "initrows": init_rows,
            "wsT": WsT,
            "wmT": WmT,
            "bsrow": bs.reshape(1, H).astype(np.float16),
            "bmrow": bm.reshape(1, MSG).astype(np.float16),
            "sidx": np.ascontiguousarray(sched["sidx"][c]),
            "fidx": np.ascontiguousarray(sched["fidx"][c]),
            "fridx": np.ascontiguousarray(sched["fridx"][c]),
        })

    res = bass_utils.run_bass_kernel_spmd(
        nc, in_maps, core_ids=list(range(NCORES)))

    diag = np.zeros((N, H), np.float32)
    for c in range(NCORES):
        rs = sched["runs_of"][c]
        diag[rs] = np.asarray(res.results[c]["diag"][:len(rs)], np.float32)

    logits = diag @ Wd.T + bd
    mx = logits.max(axis=-1, keepdims=True)
    z = logits - mx
    lse = np.log(np.exp(z).sum(axis=-1, keepdims=True))
    return (z - lse).astype(np.float32)
